# revision 17
# baseline (speedup 1.0000x reference)
"""Trainium2 Bass kernel for nn_BlockLinear forward.

Computes y[b, o] = sum_k exp(log_weight[o, k]) * x[b, o*K + k]
for x [16384, 8192] fp32, log_weight [1024, 8] fp32.

Strategy: data-parallel over batch across 8 NeuronCores (2048 rows each).
Per core, 16 tiles of [128, 8192] stream through SBUF.  The fused
multiply + grouped-reduce runs as ONE custom DVE op per chunk:

    S[p, t] = cumsum_t(x[p, t] * w[t])        (scan(ADD, Src0*Src1), II=1)

The scan is SEGMENTED in hardware: a hand-grafted SUB_DIM_DONE step
state in the uop FSM drops the CURR feedback for exactly one element at
every page boundary of in0's [P, G, K] access pattern, resetting the
running sum per group of K.  The OUTPUT access pattern has innermost
stride 0 over each group: all K writes land on one address and the last
(the completed group sum) survives — one instruction per chunk produces
the finished y chunk, contiguous and compact.

Perf model (from the ntff profile): the core's 16 DMA engines move
1KiB packets at ~38-40ns each (25-27 GB/s/engine, ~430 GB/s/core
peak; stretched to ~21-23 GB/s when the chip's HBM is contended) and
are the binding resource — they run ~98% busy for the whole x stream.
Total engine work/core = 64 MiB x + y bytes.  Design rules learned
from failed variants (lockstep collapse when the HWDGE queue runs
dry): (1) keep per-DMA-instruction engine work large — full-tile
loads are 10.3us/instruction, amortizing the ~0.7us handoff bubble;
(2) keep (x_bufs x per-buf work) well above the scan-start delay so a
load at the FIFO head never waits on a recent scan.  Optimizations
over the f32/monolithic-tail baseline (~208-212us):
  - y is stored as bf16 (rel err 1.8e-3 << 2e-2 budget), halving the
    y engine time; one full-row store per tile keeps store rows at
    2KiB so the 1KiB DMA packets stay full;
  - x_bufs=5 (vs 4) keeps the HWDGE queue deep through slow-broadcast
    startups and contended phases — zero mid-stream engine gaps;
  - the last 3 tiles load as quarter chunks (the final tile ends in
    eighths) into work-pool slot column ranges: at slot distance 5
    the reuse wait targets a scan ~50us back (no FIFO head-of-line
    stall), quarter scans track quarter arrivals so the DVE lag from
    the last full 8.7us scan decays across the taper, and the
    post-stream drain shrinks from ~16us to ~5.5us (residual: ~1.2us
    DVE lag + 1.2us last scan + ~2.1us store issue handoff+packets);
  - loads ride the Sync HWDGE queue and stores the ScalarE HWDGE
    queue so store sem-waits never block load issue (HWDGE is FIFO
    per issuing engine).

Measured on the 8 axon trn2 cores, interleaved A/B vs the baseline
over 13 rounds: baseline 202-239us (mean ~222), this kernel 185-222us
(mean ~206) — ~17-26us faster at equal HBM-contention phase (4 of our
cores share a chip with a neighboring tenant, so absolute times swing
run to run).  Good-phase budget: 9.5us fixed preamble (DVE table
barrier + HWDGE pickup) + ~167.5us engine-limited stream + 5.5us
drain.  Best observed 184.6us via test.py, rel err 1.8e-3.
"""

import numpy as np

B = 16384
IN_F = 8192
OUT_F = 1024
K = 8
N_CORES = 8
P = 128

_CACHE = {}

_OP_NAME = "SEGSUM_MUL_SCAN_ANT"
_OP2_NAME = "SEGSUM8_RESET_ANT"


def _build_seg_uops(spec, ver):
    """Lower scan(ADD, Src0*Src1) then graft a SUB_DIM_DONE step state that
    drops the CURR feedback for one element — an exact segmented scan that
    resets at every page boundary of in0's [P, S, N] access pattern."""
    import dataclasses

    from concourse import dve_spec as ds
    from concourse.dve_uop import Trigger

    spec_h = ds._hoist_stream_invariant_ops(spec)
    scans = ds._collect(spec_h.body, ds.Scan)
    latches = ds._collect(spec_h.body, ds.Latch)
    placement = ds._build_placement(
        spec_h, scans, ds.N_STAGES[ver], ds.N_LANES[ver]
    )
    states = ds._build_state_machine(spec_h, scans, latches, placement)
    d = placement.node_stage[scans[0]]
    steady_idx = len(states) - 1
    step_idx = steady_idx + 1
    steady = states[steady_idx]
    states[steady_idx] = dataclasses.replace(
        steady,
        trigger=(Trigger.SRC_TENSOR_DONE, Trigger.SUB_DIM_DONE, Trigger.NONE),
        next=(0, step_idx, 0),
    )
    states.append(
        dataclasses.replace(
            steady,
            overrides={
                **steady.overrides,
                d: ds._Stage(ds.AluOp.BYPASS, scans[0].expr),
            },
            trigger=(Trigger.SRC_TENSOR_DONE, Trigger.SUB_DIM_DONE, Trigger.COUNT),
            next=(0, step_idx, steady_idx),
            repeat=1,
        )
    )
    uops = [ds._assemble(st) for st in states]
    for u in uops:
        u.validate(ver)
    return uops


def _register_seg_op():
    """Register the segmented multiply-scan (page-reset) custom DVE op."""
    import dataclasses

    from concourse import dve_ops
    from concourse.dve_spec import AluOp, Spec, Src0, Src1, scan
    from concourse.dve_uop import DveOpSpec

    for op in dve_ops.OPS:
        if op.name == _OP2_NAME:
            return op

    def _ref(in0, in1, s0, s1, imm2):
        p = (
            np.asarray(in0, np.float32)
            * np.asarray(in1, np.float32).reshape(np.asarray(in0).shape)
        ).astype(np.float32)
        return np.cumsum(p, axis=-1, dtype=np.float32)

    spec = Spec(body=scan(AluOp.ADD, Src0 * Src1), reference=_ref)

    @dataclasses.dataclass(frozen=True)
    class _SegDveOp(dve_ops.DveOp):
        def compile(self, ver):
            key = (self.name, ver)
            cached = dve_ops._COMPILE_CACHE.get(key)
            if cached is not None:
                return cached
            result = DveOpSpec(
                name=self.name,
                opcode=dve_ops.get_dve_sub_opcode(self.name),
                uops=_build_seg_uops(self.spec, ver),
                rd1_en=True,
            )
            got = result.sha(ver)
            if self.uops_sha.get(ver) != got:
                raise ValueError(f"{self.name}: uop drift {got}")
            dve_ops._COMPILE_CACHE[key] = result
            return result

    row = dve_ops._CUSTOM_DVE_ROW_BASE + len(dve_ops.OPS)
    shas = {}
    for ver in ("v3", "v4"):
        s = DveOpSpec(
            name=_OP2_NAME, opcode=row, uops=_build_seg_uops(spec, ver), rd1_en=True
        )
        shas[ver] = s.sha(ver)
    op = _SegDveOp(_OP2_NAME, spec, subdim=True, uops_sha=shas)
    dve_ops.OPS.append(op)
    dve_ops.CUSTOM_DVE_SPECS[_OP2_NAME] = spec
    dve_ops._SUB_OPCODE_FOR_NAME[_OP2_NAME] = row
    return op


def _register_custom_op():
    """Register scan(ADD, Src0*Src1) as a custom DVE op (runtime-local)."""
    from concourse import dve_ops
    from concourse.dve_spec import AluOp, Spec, Src0, Src1, _has_src1, lower, scan
    from concourse.dve_uop import DveOpSpec

    for op in dve_ops.OPS:
        if op.name == _OP_NAME:
            return op

    def _ref(in0, in1, s0, s1, imm2):
        p = (np.asarray(in0, np.float32) * np.asarray(in1, np.float32)).astype(
            np.float32
        )
        shp = p.shape
        return (
            np.cumsum(p.reshape(shp[0], -1), axis=1, dtype=np.float32).reshape(shp)
        )

    spec = Spec(body=scan(AluOp.ADD, Src0 * Src1), reference=_ref)
    row = dve_ops._CUSTOM_DVE_ROW_BASE + len(dve_ops.OPS)
    shas = {}
    for ver in ("v3", "v4"):
        s = DveOpSpec(
            name=_OP_NAME, opcode=row, uops=lower(spec, ver=ver), rd1_en=_has_src1(spec)
        )
        shas[ver] = s.sha(ver)
    op = dve_ops.DveOp(_OP_NAME, spec, subdim=False, uops_sha=shas)
    dve_ops.OPS.append(op)
    dve_ops.CUSTOM_DVE_SPECS[_OP_NAME] = spec
    dve_ops._SUB_OPCODE_FOR_NAME[_OP_NAME] = row
    return op


def _build(b_shard, in_f, out_f, n_cores, x_bufs=5, quarters=4, y_bufs=3,
           y_bf16=True, n_tail=3):
    """Build + compile the per-core Bass module (SPMD across n_cores).

    Tiles 0..n-3 load as FULL [128, in_f] tiles (10.3us of DMA-engine
    work per instruction amortizes the ~0.7us per-instruction handoff
    bubble; 4 bufs = ~41us of buffer-reuse runway keeps the HWDGE queue
    deep).  The last n_tail tiles load as column quarters into a
    DEDICATED pool — their loads never wait on recent scans, each
    quarter is scanned on arrival, so the post-stream drain is a couple
    of 2.3us scans + one store instead of a ~16us serial tail.  y
    accumulates per tile in SBUF (bf16) and is stored once per row-tile
    so store rows stay 2KiB (full 1KiB DMA packets).
    """
    from concourse import bacc, tile, mybir

    _register_custom_op()
    op2 = _register_seg_op()

    k = K
    n_tiles = b_shard // P
    qw = in_f // quarters  # quarter width (multiple of K), = w bcast granule
    f32 = mybir.dt.float32
    y_dt = mybir.dt.bfloat16 if y_bf16 else f32

    nc = bacc.Bacc(
        "TRN2",
        target_bir_lowering=False,
        debug=False,
        enable_asserts=True,
        num_devices=n_cores,
    )
    x_d = nc.dram_tensor("x", [b_shard, in_f], f32, kind="ExternalInput")
    w_d = nc.dram_tensor("w", [1, in_f], f32, kind="ExternalInput")
    y_d = nc.dram_tensor("y", [b_shard, out_f], y_dt, kind="ExternalOutput")

    with tile.TileContext(nc) as tc:
        with (
            tc.tile_pool(name="consts", bufs=1) as cpool,
            tc.tile_pool(name="work", bufs=x_bufs) as pool,
            tc.tile_pool(name="outs", bufs=y_bufs) as ypool,
        ):
            wb = cpool.tile([P, in_f], f32, tag="w")
            # w first in the Sync HWDGE FIFO: its 32KB completes ~5us
            # earlier than via SWDGE (GpSimd's preamble delays emission),
            # and it only displaces x0's issue by ~0.7us.
            nc.sync.dma_start(out=wb[0:1, :], in_=w_d[:])
            for h in range(quarters):
                # quarter-granular broadcast: tile 0's quarter scans gate
                # on their own partial broadcast (Tile deps are AP-range
                # based), so compute starts before the full wb is ready.
                nc.gpsimd.partition_broadcast(
                    wb[:, h * qw : (h + 1) * qw], wb[0:1, h * qw : (h + 1) * qw]
                )

            def scan_chunk(yt, xap, c0, cw):
                """Scan columns [c0, c0+cw) from AP xap into yt's groups.

                One instruction per chunk: segmented multiply-scan with a
                hardware page reset (SUB_DIM_DONE step state) over in0's
                [P, cg, K] access pattern.  The out AP has innermost
                stride 0 over each group's K elements, so the last write
                (the completed group sum) survives, laid out contiguously;
                the DVE output converter rounds the surviving f32 sum to
                bf16 on write."""
                cg = cw // k
                y_view = (
                    yt[:, c0 // k : (c0 + cw) // k]
                    .rearrange("p (g o) -> p g o", o=1)
                    .broadcast_to([P, cg, k])
                )
                nc.vector._custom_dve(
                    op2,
                    out=y_view,
                    in0=xap.rearrange("p (g kk) -> p g kk", kk=k),
                    in1=wb[:, c0 : c0 + cw],
                )

            for i in range(n_tiles):
                rows = slice(i * P, (i + 1) * P)
                yt = ypool.tile([P, out_f], y_dt, tag="y")
                if i >= n_tiles - n_tail:
                    # tail tiles: chunked loads into column ranges of a
                    # work-pool slot.  At slot distance x_bufs=5 the
                    # buffer-reuse wait targets a scan ~50us back — no
                    # head-of-line stall even in a contended HBM phase —
                    # and quarter granularity keeps semaphore reuse-guard
                    # pressure low.  Quarter scans (2.3us) track quarter
                    # arrivals (2.5us), so the DVE lag from the last full
                    # 8.7us scan decays to ~0 over the first tail tile;
                    # the final tile ends in eighth chunks so the
                    # post-stream drain is a 1.2us scan + one store.
                    xt = pool.tile([P, in_f], f32, tag="x")
                    if i == n_tiles - 1:
                        widths = [qw, qw, qw // 2, qw // 2, qw // 2, qw // 2]
                    else:
                        widths = [qw] * quarters
                    c0 = 0
                    for cw in widths:
                        nc.sync.dma_start(
                            out=xt[:, c0 : c0 + cw],
                            in_=x_d[rows, c0 : c0 + cw],
                        )
                        scan_chunk(yt, xt[:, c0 : c0 + cw], c0, cw)
                        c0 += cw
                else:
                    xt = pool.tile([P, in_f], f32, tag="x")
                    if i == 0:
                        nc.sync.dma_start(out=xt[:], in_=x_d[rows, :])
                        # quarter-scans against matching wb ranges: each
                        # gates on its own partial broadcast, starting
                        # compute earlier (no extra bytes moved)
                        for q in range(quarters):
                            scan_chunk(yt, xt[:, q * qw : (q + 1) * qw], q * qw, qw)
                    else:
                        nc.sync.dma_start(out=xt[:], in_=x_d[rows, :])
                        scan_chunk(yt, xt[:], 0, in_f)
                # One full-row store per tile (2KiB/partition in bf16 keeps
                # the 1KiB DMA packets full).  It rides the ScalarE HWDGE
                # queue so its semaphore waits never block load issue.
                nc.scalar.dma_start(out=y_d[rows, :], in_=yt[:])
    nc.compile()
    return nc


def _prep_weights(log_weight, out_f, k):
    w = np.exp(np.asarray(log_weight, np.float64)).reshape(1, -1)  # [1, out_f*k]
    return np.ascontiguousarray(w, dtype=np.float32)


def kernel(x, log_weight):
    from concourse import bass_utils

    x = np.ascontiguousarray(np.asarray(x, dtype=np.float32))
    assert x.shape == (B, IN_F), x.shape
    b_shard = B // N_CORES

    if "nc" not in _CACHE:
        _CACHE["nc"] = _build(b_shard, IN_F, OUT_F, N_CORES)
    nc = _CACHE["nc"]

    wb = _prep_weights(log_weight, OUT_F, K)
    in_maps = [
        {"x": x[i * b_shard : (i + 1) * b_shard], "w": wb}
        for i in range(N_CORES)
    ]
    res = bass_utils.run_bass_kernel_spmd(nc, in_maps, core_ids=list(range(N_CORES)))
    y = np.concatenate(
        [np.asarray(res.results[i]["y"]) for i in range(N_CORES)], axis=0
    ).astype(np.float32)
    return y


# revision 18
# speedup vs baseline: 1.0829x; 1.0829x over previous
"""Trainium2 Bass kernel for nn_BlockLinear forward.

Computes y[b, o] = sum_k exp(log_weight[o, k]) * x[b, o*K + k]
for x [16384, 8192] fp32, log_weight [1024, 8] fp32.

Strategy: data-parallel over batch across 8 NeuronCores (2048 rows each).
Per core, 16 tiles of [128, 8192] stream through SBUF.  The fused
multiply + grouped-reduce runs as ONE custom DVE op per chunk:

    S[p, t] = cumsum_t(x[p, t] * w[t])        (scan(ADD, Src0*Src1), II=1)

The scan is SEGMENTED in hardware: a hand-grafted SUB_DIM_DONE step
state in the uop FSM drops the CURR feedback for exactly one element at
every page boundary of in0's [P, G, K] access pattern, resetting the
running sum per group of K.  The OUTPUT access pattern has innermost
stride 0 over each group: all K writes land on one address and the last
(the completed group sum) survives — one instruction per chunk produces
the finished y chunk, contiguous and compact.

Perf model (from the ntff profile): the core's 16 DMA engines move
1KiB packets at ~38-40ns each (25-27 GB/s/engine, ~430 GB/s/core
peak; stretched to ~21-23 GB/s when the chip's HBM is contended) and
are the binding resource — they run ~98% busy for the whole x stream.
Total engine work/core = 64 MiB x + y bytes.  Design rules learned
from failed variants (lockstep collapse when the HWDGE queue runs
dry): (1) keep per-DMA-instruction engine work large — full-tile
loads are 10.3us/instruction, amortizing the ~0.7us handoff bubble;
(2) keep (x_bufs x per-buf work) well above the scan-start delay so a
load at the FIFO head never waits on a recent scan.  Optimizations
over the f32/monolithic-tail baseline (~208-212us):
  - y is stored as bf16 (rel err 1.8e-3 << 2e-2 budget), halving the
    y engine time; one full-row store per tile keeps store rows at
    2KiB so the 1KiB DMA packets stay full;
  - x_bufs=5 (vs 4) keeps the HWDGE queue deep through slow-broadcast
    startups and contended phases — zero mid-stream engine gaps;
  - the last 3 tiles load as quarter chunks (the final tile ends in
    eighths) into work-pool slot column ranges: at slot distance 5
    the reuse wait targets a scan ~50us back (no FIFO head-of-line
    stall), quarter scans track quarter arrivals so the DVE lag from
    the last full 8.7us scan decays across the taper, and the
    post-stream drain shrinks from ~16us to ~5.5us (residual: ~1.2us
    DVE lag + 1.2us last scan + ~2.1us store issue handoff+packets);
  - loads ride the Sync HWDGE queue and stores the ScalarE HWDGE
    queue so store sem-waits never block load issue (HWDGE is FIFO
    per issuing engine).

Measured on the 8 axon trn2 cores, interleaved A/B vs the baseline
over 13 rounds: baseline 202-239us (mean ~222), this kernel 185-222us
(mean ~206) — ~17-26us faster at equal HBM-contention phase (4 of our
cores share a chip with a neighboring tenant, so absolute times swing
run to run).  Good-phase budget: 9.5us fixed preamble (DVE table
barrier + HWDGE pickup) + ~167.5us engine-limited stream + 5.5us
drain.  Best observed 184.6us via test.py, rel err 1.8e-3.
"""

import numpy as np

B = 16384
IN_F = 8192
OUT_F = 1024
K = 8
N_CORES = 8
P = 128

_CACHE = {}

_OP_NAME = "SEGSUM_MUL_SCAN_ANT"
_OP2_NAME = "SEGSUM8_RESET_ANT"


def _build_seg_uops(spec, ver):
    """Lower scan(ADD, Src0*Src1) then graft a SUB_DIM_DONE step state that
    drops the CURR feedback for one element — an exact segmented scan that
    resets at every page boundary of in0's [P, S, N] access pattern."""
    import dataclasses

    from concourse import dve_spec as ds
    from concourse.dve_uop import Trigger

    spec_h = ds._hoist_stream_invariant_ops(spec)
    scans = ds._collect(spec_h.body, ds.Scan)
    latches = ds._collect(spec_h.body, ds.Latch)
    placement = ds._build_placement(
        spec_h, scans, ds.N_STAGES[ver], ds.N_LANES[ver]
    )
    states = ds._build_state_machine(spec_h, scans, latches, placement)
    d = placement.node_stage[scans[0]]
    steady_idx = len(states) - 1
    step_idx = steady_idx + 1
    steady = states[steady_idx]
    states[steady_idx] = dataclasses.replace(
        steady,
        trigger=(Trigger.SRC_TENSOR_DONE, Trigger.SUB_DIM_DONE, Trigger.NONE),
        next=(0, step_idx, 0),
    )
    states.append(
        dataclasses.replace(
            steady,
            overrides={
                **steady.overrides,
                d: ds._Stage(ds.AluOp.BYPASS, scans[0].expr),
            },
            trigger=(Trigger.SRC_TENSOR_DONE, Trigger.SUB_DIM_DONE, Trigger.COUNT),
            next=(0, step_idx, steady_idx),
            repeat=1,
        )
    )
    uops = [ds._assemble(st) for st in states]
    for u in uops:
        u.validate(ver)
    return uops


def _register_seg_op():
    """Register the segmented multiply-scan (page-reset) custom DVE op."""
    import dataclasses

    from concourse import dve_ops
    from concourse.dve_spec import AluOp, Spec, Src0, Src1, scan
    from concourse.dve_uop import DveOpSpec

    for op in dve_ops.OPS:
        if op.name == _OP2_NAME:
            return op

    def _ref(in0, in1, s0, s1, imm2):
        p = (
            np.asarray(in0, np.float32)
            * np.asarray(in1, np.float32).reshape(np.asarray(in0).shape)
        ).astype(np.float32)
        return np.cumsum(p, axis=-1, dtype=np.float32)

    spec = Spec(body=scan(AluOp.ADD, Src0 * Src1), reference=_ref)

    @dataclasses.dataclass(frozen=True)
    class _SegDveOp(dve_ops.DveOp):
        def compile(self, ver):
            key = (self.name, ver)
            cached = dve_ops._COMPILE_CACHE.get(key)
            if cached is not None:
                return cached
            result = DveOpSpec(
                name=self.name,
                opcode=dve_ops.get_dve_sub_opcode(self.name),
                uops=_build_seg_uops(self.spec, ver),
                rd1_en=True,
            )
            got = result.sha(ver)
            if self.uops_sha.get(ver) != got:
                raise ValueError(f"{self.name}: uop drift {got}")
            dve_ops._COMPILE_CACHE[key] = result
            return result

    row = dve_ops._CUSTOM_DVE_ROW_BASE + len(dve_ops.OPS)
    shas = {}
    for ver in ("v3", "v4"):
        s = DveOpSpec(
            name=_OP2_NAME, opcode=row, uops=_build_seg_uops(spec, ver), rd1_en=True
        )
        shas[ver] = s.sha(ver)
    op = _SegDveOp(_OP2_NAME, spec, subdim=True, uops_sha=shas)
    dve_ops.OPS.append(op)
    dve_ops.CUSTOM_DVE_SPECS[_OP2_NAME] = spec
    dve_ops._SUB_OPCODE_FOR_NAME[_OP2_NAME] = row
    return op


def _register_custom_op():
    """Register scan(ADD, Src0*Src1) as a custom DVE op (runtime-local)."""
    from concourse import dve_ops
    from concourse.dve_spec import AluOp, Spec, Src0, Src1, _has_src1, lower, scan
    from concourse.dve_uop import DveOpSpec

    for op in dve_ops.OPS:
        if op.name == _OP_NAME:
            return op

    def _ref(in0, in1, s0, s1, imm2):
        p = (np.asarray(in0, np.float32) * np.asarray(in1, np.float32)).astype(
            np.float32
        )
        shp = p.shape
        return (
            np.cumsum(p.reshape(shp[0], -1), axis=1, dtype=np.float32).reshape(shp)
        )

    spec = Spec(body=scan(AluOp.ADD, Src0 * Src1), reference=_ref)
    row = dve_ops._CUSTOM_DVE_ROW_BASE + len(dve_ops.OPS)
    shas = {}
    for ver in ("v3", "v4"):
        s = DveOpSpec(
            name=_OP_NAME, opcode=row, uops=lower(spec, ver=ver), rd1_en=_has_src1(spec)
        )
        shas[ver] = s.sha(ver)
    op = dve_ops.DveOp(_OP_NAME, spec, subdim=False, uops_sha=shas)
    dve_ops.OPS.append(op)
    dve_ops.CUSTOM_DVE_SPECS[_OP_NAME] = spec
    dve_ops._SUB_OPCODE_FOR_NAME[_OP_NAME] = row
    return op


def _build(b_shard, in_f, out_f, n_cores, x_bufs=5, quarters=4, y_bufs=3,
           y_bf16=True, n_tail=3):
    """Build + compile the per-core Bass module (SPMD across n_cores).

    Tiles 0..n-3 load as FULL [128, in_f] tiles (10.3us of DMA-engine
    work per instruction amortizes the ~0.7us per-instruction handoff
    bubble; 4 bufs = ~41us of buffer-reuse runway keeps the HWDGE queue
    deep).  The last n_tail tiles load as column quarters into a
    DEDICATED pool — their loads never wait on recent scans, each
    quarter is scanned on arrival, so the post-stream drain is a couple
    of 2.3us scans + one store instead of a ~16us serial tail.  y
    accumulates per tile in SBUF (bf16) and is stored once per row-tile
    so store rows stay 2KiB (full 1KiB DMA packets).
    """
    from concourse import bacc, tile, mybir

    _register_custom_op()
    op2 = _register_seg_op()

    k = K
    n_tiles = b_shard // P
    qw = in_f // quarters  # quarter width (multiple of K), = w bcast granule
    f32 = mybir.dt.float32
    y_dt = mybir.dt.bfloat16 if y_bf16 else f32

    nc = bacc.Bacc(
        "TRN2",
        target_bir_lowering=False,
        debug=False,
        enable_asserts=True,
        num_devices=n_cores,
    )
    x_d = nc.dram_tensor("x", [b_shard, in_f], f32, kind="ExternalInput")
    w_d = nc.dram_tensor("w", [1, in_f], f32, kind="ExternalInput")
    y_d = nc.dram_tensor("y", [b_shard, out_f], y_dt, kind="ExternalOutput")

    with tile.TileContext(nc) as tc:
        with (
            tc.tile_pool(name="consts", bufs=1) as cpool,
            tc.tile_pool(name="work", bufs=x_bufs) as pool,
            tc.tile_pool(name="outs", bufs=y_bufs) as ypool,
        ):
            wb = cpool.tile([P, in_f], f32, tag="w")
            # w first in the Sync HWDGE FIFO: its 32KB completes ~5us
            # earlier than via SWDGE (GpSimd's preamble delays emission),
            # and it only displaces x0's issue by ~0.7us.
            nc.sync.dma_start(out=wb[0:1, :], in_=w_d[:])
            for h in range(quarters):
                # quarter-granular broadcast: tile 0's quarter scans gate
                # on their own partial broadcast (Tile deps are AP-range
                # based), so compute starts before the full wb is ready.
                nc.gpsimd.partition_broadcast(
                    wb[:, h * qw : (h + 1) * qw], wb[0:1, h * qw : (h + 1) * qw]
                )

            def scan_chunk(yt, xap, c0, cw):
                """Scan columns [c0, c0+cw) from AP xap into yt's groups.

                One instruction per chunk: segmented multiply-scan with a
                hardware page reset (SUB_DIM_DONE step state) over in0's
                [P, cg, K] access pattern.  The out AP has innermost
                stride 0 over each group's K elements, so the last write
                (the completed group sum) survives, laid out contiguously;
                the DVE output converter rounds the surviving f32 sum to
                bf16 on write."""
                cg = cw // k
                y_view = (
                    yt[:, c0 // k : (c0 + cw) // k]
                    .rearrange("p (g o) -> p g o", o=1)
                    .broadcast_to([P, cg, k])
                )
                nc.vector._custom_dve(
                    op2,
                    out=y_view,
                    in0=xap.rearrange("p (g kk) -> p g kk", kk=k),
                    in1=wb[:, c0 : c0 + cw],
                )

            for i in range(n_tiles):
                rows = slice(i * P, (i + 1) * P)
                yt = ypool.tile([P, out_f], y_dt, tag="y")
                if i >= n_tiles - n_tail:
                    # tail tiles: chunked loads into column ranges of a
                    # work-pool slot.  At slot distance x_bufs=5 the
                    # buffer-reuse wait targets a scan ~50us back — no
                    # head-of-line stall even in a contended HBM phase —
                    # and quarter granularity keeps semaphore reuse-guard
                    # pressure low.  Quarter scans (2.3us) track quarter
                    # arrivals (2.5us), so the DVE lag from the last full
                    # 8.7us scan decays to ~0 over the first tail tile;
                    # the final tile ends in eighth chunks so the
                    # post-stream drain is a 1.2us scan + one store.
                    xt = pool.tile([P, in_f], f32, tag="x")
                    if i == n_tiles - 1:
                        widths = [qw, qw, qw // 2, qw // 2, qw // 2, qw // 2]
                    else:
                        widths = [qw] * quarters
                    c0 = 0
                    for cw in widths:
                        nc.sync.dma_start(
                            out=xt[:, c0 : c0 + cw],
                            in_=x_d[rows, c0 : c0 + cw],
                        )
                        scan_chunk(yt, xt[:, c0 : c0 + cw], c0, cw)
                        c0 += cw
                else:
                    xt = pool.tile([P, in_f], f32, tag="x")
                    if i == 0:
                        nc.sync.dma_start(out=xt[:], in_=x_d[rows, :])
                        # quarter-scans against matching wb ranges: each
                        # gates on its own partial broadcast, starting
                        # compute earlier (no extra bytes moved)
                        for q in range(quarters):
                            scan_chunk(yt, xt[:, q * qw : (q + 1) * qw], q * qw, qw)
                    else:
                        nc.sync.dma_start(out=xt[:], in_=x_d[rows, :])
                        scan_chunk(yt, xt[:], 0, in_f)
                # One full-row store per tile (2KiB/partition in bf16 keeps
                # the 1KiB DMA packets full).  It rides the ScalarE HWDGE
                # queue so its semaphore waits never block load issue.
                # The final tile stores in two halves (1KiB rows, still
                # full packets) so the post-last-scan store is half-size.
                if i == n_tiles - 1:
                    half_g = out_f // 2
                    nc.scalar.dma_start(
                        out=y_d[rows, 0:half_g], in_=yt[:, 0:half_g]
                    )
                    nc.scalar.dma_start(
                        out=y_d[rows, half_g:out_f], in_=yt[:, half_g:out_f]
                    )
                else:
                    nc.scalar.dma_start(out=y_d[rows, :], in_=yt[:])
    nc.compile()
    return nc


def _prep_weights(log_weight, out_f, k):
    w = np.exp(np.asarray(log_weight, np.float64)).reshape(1, -1)  # [1, out_f*k]
    return np.ascontiguousarray(w, dtype=np.float32)


def kernel(x, log_weight):
    from concourse import bass_utils

    x = np.ascontiguousarray(np.asarray(x, dtype=np.float32))
    assert x.shape == (B, IN_F), x.shape
    b_shard = B // N_CORES

    if "nc" not in _CACHE:
        _CACHE["nc"] = _build(b_shard, IN_F, OUT_F, N_CORES)
    nc = _CACHE["nc"]

    wb = _prep_weights(log_weight, OUT_F, K)
    in_maps = [
        {"x": x[i * b_shard : (i + 1) * b_shard], "w": wb}
        for i in range(N_CORES)
    ]
    res = bass_utils.run_bass_kernel_spmd(nc, in_maps, core_ids=list(range(N_CORES)))
    y = np.concatenate(
        [np.asarray(res.results[i]["y"]) for i in range(N_CORES)], axis=0
    ).astype(np.float32)
    return y


# revision 19
# speedup vs baseline: 1.0954x; 1.0116x over previous
"""Trainium2 Bass kernel for nn_BlockLinear forward.

Computes y[b, o] = sum_k exp(log_weight[o, k]) * x[b, o*K + k]
for x [16384, 8192] fp32, log_weight [1024, 8] fp32.

Strategy: data-parallel over batch across 8 NeuronCores (2048 rows each).
Per core, 16 tiles of [128, 8192] stream through SBUF.  The fused
multiply + grouped-reduce runs as ONE custom DVE op per chunk:

    S[p, t] = cumsum_t(x[p, t] * w[t])        (scan(ADD, Src0*Src1), II=1)

The scan is SEGMENTED in hardware: a hand-grafted SUB_DIM_DONE step
state in the uop FSM drops the CURR feedback for exactly one element at
every page boundary of in0's [P, G, K] access pattern, resetting the
running sum per group of K.  The OUTPUT access pattern has innermost
stride 0 over each group: all K writes land on one address and the last
(the completed group sum) survives — one instruction per chunk produces
the finished y chunk, contiguous and compact.

Perf model (from the ntff profile): the core's 16 DMA engines move
1KiB packets at ~38-40ns each (25-27 GB/s/engine, ~430 GB/s/core
peak; stretched to ~21-23 GB/s when the chip's HBM is contended) and
are the binding resource — they run ~98% busy for the whole x stream.
Total engine work/core = 64 MiB x + y bytes.  Design rules learned
from failed variants (lockstep collapse when the HWDGE queue runs
dry): (1) keep per-DMA-instruction engine work large — full-tile
loads are 10.3us/instruction, amortizing the ~0.7us handoff bubble;
(2) keep (x_bufs x per-buf work) well above the scan-start delay so a
load at the FIFO head never waits on a recent scan.  Optimizations
over the f32/monolithic-tail baseline (~208-212us):
  - y is stored as bf16 (rel err 1.8e-3 << 2e-2 budget), halving the
    y engine time; one full-row store per tile keeps store rows at
    2KiB so the 1KiB DMA packets stay full;
  - x_bufs=5 (vs 4) keeps the HWDGE queue deep through slow-broadcast
    startups and contended phases — zero mid-stream engine gaps;
  - the last 3 tiles load as quarter chunks (the final tile ends in
    eighths) into work-pool slot column ranges: at slot distance 5
    the reuse wait targets a scan ~50us back (no FIFO head-of-line
    stall), quarter scans track quarter arrivals so the DVE lag from
    the last full 8.7us scan decays across the taper, and the final
    tile stores in two halves (1KiB rows, still full packets) so the
    post-last-scan store is half-size — the post-stream drain
    shrinks from ~16us to ~3.4us;
  - loads ride the Sync HWDGE queue and stores the ScalarE HWDGE
    queue so store sem-waits never block load issue (HWDGE is FIFO
    per issuing engine).

Measured on the 8 axon trn2 cores, interleaved A/B vs the baseline
over 13 rounds: baseline 202-239us (mean ~222), this kernel 185-222us
(mean ~206) — ~17-26us faster at equal HBM-contention phase (4 of our
cores share a chip with a neighboring tenant, so absolute times swing
run to run).  Good-phase budget: 9.5us fixed preamble (DVE table
barrier + HWDGE pickup) + ~167.5us engine-limited stream + 5.5us
drain.  Best observed 184.6us via test.py, rel err 1.8e-3.
"""

import numpy as np

B = 16384
IN_F = 8192
OUT_F = 1024
K = 8
N_CORES = 8
P = 128

_CACHE = {}

_OP_NAME = "SEGSUM_MUL_SCAN_ANT"
_OP2_NAME = "SEGSUM8_RESET_ANT"


def _build_seg_uops(spec, ver):
    """Lower scan(ADD, Src0*Src1) then graft a SUB_DIM_DONE step state that
    drops the CURR feedback for one element — an exact segmented scan that
    resets at every page boundary of in0's [P, S, N] access pattern."""
    import dataclasses

    from concourse import dve_spec as ds
    from concourse.dve_uop import Trigger

    spec_h = ds._hoist_stream_invariant_ops(spec)
    scans = ds._collect(spec_h.body, ds.Scan)
    latches = ds._collect(spec_h.body, ds.Latch)
    placement = ds._build_placement(
        spec_h, scans, ds.N_STAGES[ver], ds.N_LANES[ver]
    )
    states = ds._build_state_machine(spec_h, scans, latches, placement)
    d = placement.node_stage[scans[0]]
    steady_idx = len(states) - 1
    step_idx = steady_idx + 1
    steady = states[steady_idx]
    states[steady_idx] = dataclasses.replace(
        steady,
        trigger=(Trigger.SRC_TENSOR_DONE, Trigger.SUB_DIM_DONE, Trigger.NONE),
        next=(0, step_idx, 0),
    )
    states.append(
        dataclasses.replace(
            steady,
            overrides={
                **steady.overrides,
                d: ds._Stage(ds.AluOp.BYPASS, scans[0].expr),
            },
            trigger=(Trigger.SRC_TENSOR_DONE, Trigger.SUB_DIM_DONE, Trigger.COUNT),
            next=(0, step_idx, steady_idx),
            repeat=1,
        )
    )
    uops = [ds._assemble(st) for st in states]
    for u in uops:
        u.validate(ver)
    return uops


def _register_seg_op():
    """Register the segmented multiply-scan (page-reset) custom DVE op."""
    import dataclasses

    from concourse import dve_ops
    from concourse.dve_spec import AluOp, Spec, Src0, Src1, scan
    from concourse.dve_uop import DveOpSpec

    for op in dve_ops.OPS:
        if op.name == _OP2_NAME:
            return op

    def _ref(in0, in1, s0, s1, imm2):
        p = (
            np.asarray(in0, np.float32)
            * np.asarray(in1, np.float32).reshape(np.asarray(in0).shape)
        ).astype(np.float32)
        return np.cumsum(p, axis=-1, dtype=np.float32)

    spec = Spec(body=scan(AluOp.ADD, Src0 * Src1), reference=_ref)

    @dataclasses.dataclass(frozen=True)
    class _SegDveOp(dve_ops.DveOp):
        def compile(self, ver):
            key = (self.name, ver)
            cached = dve_ops._COMPILE_CACHE.get(key)
            if cached is not None:
                return cached
            result = DveOpSpec(
                name=self.name,
                opcode=dve_ops.get_dve_sub_opcode(self.name),
                uops=_build_seg_uops(self.spec, ver),
                rd1_en=True,
            )
            got = result.sha(ver)
            if self.uops_sha.get(ver) != got:
                raise ValueError(f"{self.name}: uop drift {got}")
            dve_ops._COMPILE_CACHE[key] = result
            return result

    row = dve_ops._CUSTOM_DVE_ROW_BASE + len(dve_ops.OPS)
    shas = {}
    for ver in ("v3", "v4"):
        s = DveOpSpec(
            name=_OP2_NAME, opcode=row, uops=_build_seg_uops(spec, ver), rd1_en=True
        )
        shas[ver] = s.sha(ver)
    op = _SegDveOp(_OP2_NAME, spec, subdim=True, uops_sha=shas)
    dve_ops.OPS.append(op)
    dve_ops.CUSTOM_DVE_SPECS[_OP2_NAME] = spec
    dve_ops._SUB_OPCODE_FOR_NAME[_OP2_NAME] = row
    return op


def _register_custom_op():
    """Register scan(ADD, Src0*Src1) as a custom DVE op (runtime-local)."""
    from concourse import dve_ops
    from concourse.dve_spec import AluOp, Spec, Src0, Src1, _has_src1, lower, scan
    from concourse.dve_uop import DveOpSpec

    for op in dve_ops.OPS:
        if op.name == _OP_NAME:
            return op

    def _ref(in0, in1, s0, s1, imm2):
        p = (np.asarray(in0, np.float32) * np.asarray(in1, np.float32)).astype(
            np.float32
        )
        shp = p.shape
        return (
            np.cumsum(p.reshape(shp[0], -1), axis=1, dtype=np.float32).reshape(shp)
        )

    spec = Spec(body=scan(AluOp.ADD, Src0 * Src1), reference=_ref)
    row = dve_ops._CUSTOM_DVE_ROW_BASE + len(dve_ops.OPS)
    shas = {}
    for ver in ("v3", "v4"):
        s = DveOpSpec(
            name=_OP_NAME, opcode=row, uops=lower(spec, ver=ver), rd1_en=_has_src1(spec)
        )
        shas[ver] = s.sha(ver)
    op = dve_ops.DveOp(_OP_NAME, spec, subdim=False, uops_sha=shas)
    dve_ops.OPS.append(op)
    dve_ops.CUSTOM_DVE_SPECS[_OP_NAME] = spec
    dve_ops._SUB_OPCODE_FOR_NAME[_OP_NAME] = row
    return op


def _build(b_shard, in_f, out_f, n_cores, x_bufs=5, quarters=4, y_bufs=3,
           y_bf16=True, n_tail=3):
    """Build + compile the per-core Bass module (SPMD across n_cores).

    Tiles 0..n-3 load as FULL [128, in_f] tiles (10.3us of DMA-engine
    work per instruction amortizes the ~0.7us per-instruction handoff
    bubble; 4 bufs = ~41us of buffer-reuse runway keeps the HWDGE queue
    deep).  The last n_tail tiles load as column quarters into a
    DEDICATED pool — their loads never wait on recent scans, each
    quarter is scanned on arrival, so the post-stream drain is a couple
    of 2.3us scans + one store instead of a ~16us serial tail.  y
    accumulates per tile in SBUF (bf16) and is stored once per row-tile
    so store rows stay 2KiB (full 1KiB DMA packets).
    """
    from concourse import bacc, tile, mybir

    _register_custom_op()
    op2 = _register_seg_op()

    k = K
    n_tiles = b_shard // P
    qw = in_f // quarters  # quarter width (multiple of K), = w bcast granule
    f32 = mybir.dt.float32
    y_dt = mybir.dt.bfloat16 if y_bf16 else f32

    nc = bacc.Bacc(
        "TRN2",
        target_bir_lowering=False,
        debug=False,
        enable_asserts=True,
        num_devices=n_cores,
    )
    x_d = nc.dram_tensor("x", [b_shard, in_f], f32, kind="ExternalInput")
    w_d = nc.dram_tensor("w", [1, in_f], f32, kind="ExternalInput")
    y_d = nc.dram_tensor("y", [b_shard, out_f], y_dt, kind="ExternalOutput")

    with tile.TileContext(nc) as tc:
        with (
            tc.tile_pool(name="consts", bufs=1) as cpool,
            tc.tile_pool(name="work", bufs=x_bufs) as pool,
            tc.tile_pool(name="outs", bufs=y_bufs) as ypool,
        ):
            wb = cpool.tile([P, in_f], f32, tag="w")
            # w first in the Sync HWDGE FIFO: its 32KB completes ~5us
            # earlier than via SWDGE (GpSimd's preamble delays emission),
            # and it only displaces x0's issue by ~0.7us.
            nc.sync.dma_start(out=wb[0:1, :], in_=w_d[:])
            for h in range(quarters):
                # quarter-granular broadcast: tile 0's quarter scans gate
                # on their own partial broadcast (Tile deps are AP-range
                # based), so compute starts before the full wb is ready.
                nc.gpsimd.partition_broadcast(
                    wb[:, h * qw : (h + 1) * qw], wb[0:1, h * qw : (h + 1) * qw]
                )

            def scan_chunk(yt, xap, c0, cw):
                """Scan columns [c0, c0+cw) from AP xap into yt's groups.

                One instruction per chunk: segmented multiply-scan with a
                hardware page reset (SUB_DIM_DONE step state) over in0's
                [P, cg, K] access pattern.  The out AP has innermost
                stride 0 over each group's K elements, so the last write
                (the completed group sum) survives, laid out contiguously;
                the DVE output converter rounds the surviving f32 sum to
                bf16 on write."""
                cg = cw // k
                y_view = (
                    yt[:, c0 // k : (c0 + cw) // k]
                    .rearrange("p (g o) -> p g o", o=1)
                    .broadcast_to([P, cg, k])
                )
                nc.vector._custom_dve(
                    op2,
                    out=y_view,
                    in0=xap.rearrange("p (g kk) -> p g kk", kk=k),
                    in1=wb[:, c0 : c0 + cw],
                )

            for i in range(n_tiles):
                rows = slice(i * P, (i + 1) * P)
                yt = ypool.tile([P, out_f], y_dt, tag="y")
                if i >= n_tiles - n_tail:
                    # tail tiles: chunked loads into column ranges of a
                    # work-pool slot.  At slot distance x_bufs=5 the
                    # buffer-reuse wait targets a scan ~50us back — no
                    # head-of-line stall even in a contended HBM phase —
                    # and quarter granularity keeps semaphore reuse-guard
                    # pressure low.  Quarter scans (2.3us) track quarter
                    # arrivals (2.5us), so the DVE lag from the last full
                    # 8.7us scan decays to ~0 over the first tail tile;
                    # the final tile ends in eighth chunks so the
                    # post-stream drain is a 1.2us scan + one store.
                    xt = pool.tile([P, in_f], f32, tag="x")
                    if i == n_tiles - 1:
                        widths = [qw, qw, qw // 2, qw // 2, qw // 2, qw // 2]
                    else:
                        widths = [qw] * quarters
                    c0 = 0
                    for cw in widths:
                        nc.sync.dma_start(
                            out=xt[:, c0 : c0 + cw],
                            in_=x_d[rows, c0 : c0 + cw],
                        )
                        scan_chunk(yt, xt[:, c0 : c0 + cw], c0, cw)
                        c0 += cw
                else:
                    xt = pool.tile([P, in_f], f32, tag="x")
                    if i == 0:
                        nc.sync.dma_start(out=xt[:], in_=x_d[rows, :])
                        # quarter-scans against matching wb ranges: each
                        # gates on its own partial broadcast, starting
                        # compute earlier (no extra bytes moved)
                        for q in range(quarters):
                            scan_chunk(yt, xt[:, q * qw : (q + 1) * qw], q * qw, qw)
                    else:
                        nc.sync.dma_start(out=xt[:], in_=x_d[rows, :])
                        scan_chunk(yt, xt[:], 0, in_f)
                # One full-row store per tile (2KiB/partition in bf16 keeps
                # the 1KiB DMA packets full).  It rides the ScalarE HWDGE
                # queue so its semaphore waits never block load issue.
                # The final tile stores in two halves (1KiB rows, still
                # full packets) so the post-last-scan store is half-size.
                if i == n_tiles - 1:
                    half_g = out_f // 2
                    nc.scalar.dma_start(
                        out=y_d[rows, 0:half_g], in_=yt[:, 0:half_g]
                    )
                    nc.scalar.dma_start(
                        out=y_d[rows, half_g:out_f], in_=yt[:, half_g:out_f]
                    )
                else:
                    nc.scalar.dma_start(out=y_d[rows, :], in_=yt[:])
    nc.compile()
    return nc


def _prep_weights(log_weight, out_f, k):
    w = np.exp(np.asarray(log_weight, np.float64)).reshape(1, -1)  # [1, out_f*k]
    return np.ascontiguousarray(w, dtype=np.float32)


def kernel(x, log_weight):
    from concourse import bass_utils

    x = np.ascontiguousarray(np.asarray(x, dtype=np.float32))
    assert x.shape == (B, IN_F), x.shape
    b_shard = B // N_CORES

    if "nc" not in _CACHE:
        _CACHE["nc"] = _build(b_shard, IN_F, OUT_F, N_CORES)
    nc = _CACHE["nc"]

    wb = _prep_weights(log_weight, OUT_F, K)
    in_maps = [
        {"x": x[i * b_shard : (i + 1) * b_shard], "w": wb}
        for i in range(N_CORES)
    ]
    res = bass_utils.run_bass_kernel_spmd(nc, in_maps, core_ids=list(range(N_CORES)))
    y = np.concatenate(
        [np.asarray(res.results[i]["y"]) for i in range(N_CORES)], axis=0
    ).astype(np.float32)
    return y
